# revision 1
# baseline (speedup 1.0000x reference)
"""Multi-head attention block (nn_Attention) on 8 Trainium2 NeuronCores.

Reference computation (per batch element, all in fp32):
    qkv = x @ w_qkv.T + b_qkv               # [T, 3D]
    q, k, v per head (H=12, Hd=64)
    attn = softmax(q @ k.T / sqrt(Hd))
    out  = (attn @ v) @ w_proj.T + b_proj   # [T, D]

Sharding: pure data parallelism over the batch (B=8) — one batch element
per NeuronCore, weights replicated. No collectives.

Per-core kernel strategy (all matmuls in float32r — fp32 storage, TF32-like
matmul precision at full PE rate for moving dim >= 256):
  1. Load x [T,D] and w_qkv/w_proj row-major, transpose 128x128 blocks on
     the PE (fp32 transpose is exact) to get xT [D,T], wT [D,3D], wTp [D,D].
  2. qkT [1536, T] = wT_qk @ xT (features on partitions) and v_nat [T, 768]
     = x @ w_v.T, both with bias folded into the PSUM->SBUF evacuation.
     v is staged head-major as [v_h | 1] blocks of 65 columns.
  3. Per (head, 512-token query chunk): S.T tiles = kT_h.T @ qT_h on the PE
     (K=64; adjacent head pairs live at partition bases 0/64 so their
     matmuls run concurrently in distinct row groups), exp via ScalarE
     (scale=1/8 folded in, output rounded to f32r), then
     O'.T [65, tq] = [v_h | 1].T @ P.T accumulated over the 8 key tiles.
     Row 64 of O'.T is the softmax denominator; rows 0:63 are the
     unnormalized head output. Normalization multiplies by the broadcast
     reciprocal during the PSUM->SBUF evacuation into OT [D, T].
  4. out = OT.T @ wTp + b_proj, written back token-major.
"""
import os
import numpy as np

os.environ.setdefault("JAX_COMPILATION_CACHE_DIR", "/tmp/jax_neff_cache")

import concourse.bass as bass
import concourse.bacc as bacc
import concourse.tile as tile
from concourse import mybir
from concourse.masks import make_identity

F32 = mybir.dt.float32
F32R = mybir.dt.float32r

B, T, D = 8, 1024, 768
H, HD = 12, 64
SCALE = HD ** -0.5
N_CORES = 8
TT = T // 128      # 8 token tiles
DT = D // 128      # 6 feature tiles (contraction)
TQ = 512           # query chunk (matmul moving dim)
NC_CHUNKS = T // TQ  # 2


def _bcast_ap(ap_1d, parts, n):
    return bass.AP(tensor=ap_1d.tensor, offset=ap_1d.offset,
                   ap=[[0, parts], [1, n]])


def build_nc():
    nc = bacc.Bacc(trn_type="TRN2", debug=False, num_devices=N_CORES)
    x_d = nc.dram_tensor("x", (T, D), F32, kind="ExternalInput")
    wqkv_d = nc.dram_tensor("w_qkv", (3 * D, D), F32, kind="ExternalInput")
    bqkv_d = nc.dram_tensor("b_qkv", (3 * D,), F32, kind="ExternalInput")
    wproj_d = nc.dram_tensor("w_proj", (D, D), F32, kind="ExternalInput")
    bproj_d = nc.dram_tensor("b_proj", (D,), F32, kind="ExternalInput")
    out_d = nc.dram_tensor("out", (T, D), F32, kind="ExternalOutput")

    with tile.TileContext(nc) as tc:
        _body(nc, tc, x_d, wqkv_d, bqkv_d, wproj_d, bproj_d, out_d)
    nc.compile()
    return nc


def _body(nc, tc, x_d, wqkv_d, bqkv_d, wproj_d, bproj_d, out_d):
    from contextlib import ExitStack
    with ExitStack() as ctx:
        consts = ctx.enter_context(tc.tile_pool(name="consts", bufs=1))
        qkt_pool = ctx.enter_context(tc.tile_pool(name="qkt", bufs=1))
        v_pool = ctx.enter_context(tc.tile_pool(name="vst", bufs=1))
        wtp_pool = ctx.enter_context(tc.tile_pool(name="wtp", bufs=1))
        mm_ps = ctx.enter_context(tc.tile_pool(name="mmps", bufs=2, space="PSUM"))

        ident = consts.tile([128, 128], F32)
        make_identity(nc, ident[:])
        bias_qk = consts.tile([128, 12], F32)
        nc.sync.dma_start(bias_qk[:], bqkv_d[0:1536].rearrange("(t p) -> p t", p=128))
        bias_v = consts.tile([128, D], F32)
        nc.sync.dma_start(bias_v[:], _bcast_ap(bqkv_d[1536:2304], 128, D))
        bias_p = consts.tile([128, D], F32)
        nc.sync.dma_start(bias_p[:], _bcast_ap(bproj_d[0:D], 128, D))
        ones12 = consts.tile([128, 12, 1], F32)
        nc.vector.memset(ones12[:], 1.0)

        # persistent big tensors
        qkT = [qkt_pool.tile([128, T], F32R, name=f"qkT{fi}") for fi in range(12)]
        vst = [v_pool.tile([128, H, 65], F32R, name=f"vst{ti}") for ti in range(TT)]
        wTp = [wtp_pool.tile([128, D], F32R, name=f"wTp{k}") for k in range(DT)]

        # ---------------- phase 1: loads + transposes + qkv ----------------
        with ExitStack() as ctx1:
            stage = ctx1.enter_context(tc.tile_pool(name="stage", bufs=3))
            xt_pool = ctx1.enter_context(tc.tile_pool(name="xt", bufs=1))
            wtq_pool = ctx1.enter_context(tc.tile_pool(name="wtq", bufs=1))
            tp_ps = ctx1.enter_context(tc.tile_pool(name="tpps", bufs=2, space="PSUM"))

            xT = [xt_pool.tile([128, T], F32R, name=f"xT{k}") for k in range(DT)]
            wTq = [wtq_pool.tile([128, 3 * D], F32R, name=f"wTq{k}") for k in range(DT)]

            for ti in range(TT):
                xn = stage.tile([128, D], F32, tag="stage")
                nc.sync.dma_start(xn[:], x_d[128 * ti:128 * (ti + 1), :])
                for k in range(DT):
                    pt = tp_ps.tile([128, 128], F32, tag="tp")
                    nc.tensor.transpose(pt[:], xn[:, 128 * k:128 * (k + 1)], ident[:])
                    nc.vector.tensor_copy(xT[k][:, 128 * ti:128 * (ti + 1)], pt[:])
            for fi in range(18):
                wn = stage.tile([128, D], F32, tag="stage")
                nc.sync.dma_start(wn[:], wqkv_d[128 * fi:128 * (fi + 1), :])
                for k in range(DT):
                    pt = tp_ps.tile([128, 128], F32, tag="tp")
                    nc.tensor.transpose(pt[:], wn[:, 128 * k:128 * (k + 1)], ident[:])
                    nc.vector.tensor_copy(wTq[k][:, 128 * fi:128 * (fi + 1)], pt[:])
            for fi in range(DT):
                wn = stage.tile([128, D], F32, tag="stage")
                nc.sync.dma_start(wn[:], wproj_d[128 * fi:128 * (fi + 1), :])
                for k in range(DT):
                    pt = tp_ps.tile([128, 128], F32, tag="tp")
                    nc.tensor.transpose(pt[:], wn[:, 128 * k:128 * (k + 1)], ident[:])
                    nc.vector.tensor_copy(wTp[k][:, 128 * fi:128 * (fi + 1)], pt[:])

            # qkT[f, t] for q and k features (f in 0..1536)
            for fi in range(12):
                for c in range(NC_CHUNKS):
                    pq = mm_ps.tile([128, TQ], F32, tag="mm")
                    for k in range(DT):
                        nc.tensor.matmul(
                            pq[:], wTq[k][:, 128 * fi:128 * (fi + 1)],
                            xT[k][:, TQ * c:TQ * (c + 1)],
                            start=(k == 0), stop=(k == DT - 1))
                    nc.vector.tensor_scalar_add(
                        qkT[fi][:, TQ * c:TQ * (c + 1)], pq[:], bias_qk[:, fi:fi + 1])

            # v natural [t, 768] staged head-major with a ones column
            for ti in range(TT):
                for c2 in range(2):
                    pv = mm_ps.tile([128, 384], F32, tag="mm")
                    for k in range(DT):
                        nc.tensor.matmul(
                            pv[:], xT[k][:, 128 * ti:128 * (ti + 1)],
                            wTq[k][:, 1536 + 384 * c2:1536 + 384 * (c2 + 1)],
                            start=(k == 0), stop=(k == DT - 1))
                    dst = vst[ti][:, 6 * c2:6 * (c2 + 1), 0:64]
                    nc.vector.tensor_add(
                        dst,
                        pv[:].rearrange("p (h d) -> p h d", d=64),
                        bias_v[:, 384 * c2:384 * (c2 + 1)].rearrange(
                            "p (h d) -> p h d", d=64))
                nc.vector.tensor_copy(vst[ti][:, :, 64:65], ones12[:])

        # ---------------- phase 2: attention + projection ----------------
        with ExitStack() as ctx2:
            ot_pool = ctx2.enter_context(tc.tile_pool(name="ot", bufs=1))
            pt_pool = ctx2.enter_context(tc.tile_pool(name="pt", bufs=2))
            sums_pool = ctx2.enter_context(tc.tile_pool(name="sums", bufs=2))
            rsb_pool = ctx2.enter_context(tc.tile_pool(name="rsb", bufs=2))
            s_ps = ctx2.enter_context(tc.tile_pool(name="sps", bufs=1, space="PSUM"))
            o_ps = ctx2.enter_context(tc.tile_pool(name="ops", bufs=1, space="PSUM"))
            outst = ctx2.enter_context(tc.tile_pool(name="outst", bufs=3))

            OT = [ot_pool.tile([128, T], F32R, name=f"OT{k}") for k in range(DT)]

            for c in range(NC_CHUNKS):
                for hp in range(6):
                    PT = [pt_pool.tile([128, 8 * TQ], F32R, tag=f"pt{p}",
                                       name=f"PT{c}_{hp}_{p}") for p in (0, 1)]
                    # S.T matmuls, head pair interleaved (row groups 0-1 / 2-3)
                    for g in range(4):
                        sp = [s_ps.tile([128, 1024], F32, tag=f"s{p}",
                                        name=f"sps{c}_{hp}_{g}_{p}") for p in (0, 1)]
                        for tkt in range(2):
                            tk = 2 * g + tkt
                            for p in (0, 1):
                                qb = 64 * p
                                nc.tensor.matmul(
                                    sp[p][:, 512 * tkt:512 * (tkt + 1)],
                                    qkT[6 + hp][qb:qb + 64, 128 * tk:128 * (tk + 1)],
                                    qkT[hp][qb:qb + 64, TQ * c:TQ * (c + 1)],
                                    start=True, stop=True)
                        for p in (0, 1):
                            nc.scalar.activation(
                                PT[p][:, 1024 * g:1024 * (g + 1)], sp[p][:],
                                mybir.ActivationFunctionType.Exp,
                                bias=0.0, scale=float(SCALE))
                    # O'.T = [v_h | 1].T @ P.T  (accumulate over 8 key tiles)
                    for p in (0, 1):
                        h = 2 * hp + p
                        po = o_ps.tile([128, TQ], F32, tag=f"o{p}",
                                       name=f"ops{c}_{hp}_{p}")
                        for tk in range(8):
                            nc.tensor.matmul(
                                po[0:65, :], vst[tk][:, h, :],
                                PT[p][:, 512 * tk:512 * (tk + 1)],
                                start=(tk == 0), stop=(tk == 7))
                        sst = sums_pool.tile([128, TQ], F32, tag="sums",
                                             name=f"sst{c}_{hp}_{p}")
                        nc.vector.tensor_copy(sst[0:1, :], po[64:65, :])
                        nc.vector.reciprocal_approx_fast(sst[0:1, :], sst[0:1, :])
                        rsb = rsb_pool.tile([64, TQ], F32, tag="rsb",
                                            name=f"rsb{c}_{hp}_{p}")
                        nc.gpsimd.partition_broadcast(rsb[:], sst[0:1, :])
                        nc.vector.tensor_mul(
                            OT[hp][64 * p:64 * (p + 1), TQ * c:TQ * (c + 1)],
                            po[0:64, :], rsb[:])

            # projection: out[t, f] = OT.T @ wTp + b_proj
            for ti in range(TT):
                ob = outst.tile([128, D], F32, tag="ob")
                for c2 in range(2):
                    pp = mm_ps.tile([128, 384], F32, tag="mm")
                    for k in range(DT):
                        nc.tensor.matmul(
                            pp[:], OT[k][:, 128 * ti:128 * (ti + 1)],
                            wTp[k][:, 384 * c2:384 * (c2 + 1)],
                            start=(k == 0), stop=(k == DT - 1))
                    nc.vector.tensor_add(
                        ob[:, 384 * c2:384 * (c2 + 1)], pp[:],
                        bias_p[:, 384 * c2:384 * (c2 + 1)])
                nc.sync.dma_start(out_d[128 * ti:128 * (ti + 1), :], ob[:])


_CACHE = {}


def _get_runner():
    if "runner" in _CACHE:
        return _CACHE["runner"]
    import jax
    from jax.sharding import Mesh, PartitionSpec
    from jax.experimental.shard_map import shard_map
    from concourse import bass2jax
    from concourse.bass2jax import _bass_exec_p, partition_id_tensor

    nc = build_nc()
    bass2jax.install_neuronx_cc_hook()
    partition_name = nc.partition_id_tensor.name if nc.partition_id_tensor else None
    in_names, out_names, out_avals = [], [], []
    for alloc in nc.m.functions[0].allocations:
        if not isinstance(alloc, mybir.MemoryLocationSet):
            continue
        name = alloc.memorylocations[0].name
        if alloc.kind == "ExternalInput":
            if name != partition_name:
                in_names.append(name)
        elif alloc.kind == "ExternalOutput":
            out_names.append(name)
            out_avals.append(jax.core.ShapedArray(
                tuple(alloc.tensor_shape), mybir.dt.np(alloc.dtype)))
    all_in = list(in_names) + list(out_names)
    if partition_name is not None:
        all_in.append(partition_name)

    def _jbody(*args):
        operands = list(args)
        if partition_name is not None:
            operands.append(partition_id_tensor())
        return tuple(_bass_exec_p.bind(
            *operands, out_avals=tuple(out_avals), in_names=tuple(all_in),
            out_names=tuple(out_names), lowering_input_output_aliases=(),
            sim_require_finite=True, sim_require_nnan=True, nc=nc))

    devices = jax.devices()[:N_CORES]
    mesh = Mesh(np.asarray(devices), ("core",))
    nio = len(in_names) + len(out_names)
    fn = jax.jit(
        shard_map(_jbody, mesh=mesh, in_specs=(PartitionSpec("core"),) * nio,
                  out_specs=(PartitionSpec("core"),) * len(out_names),
                  check_rep=False),
        keep_unused=True)
    _CACHE["runner"] = (fn, in_names, out_names, out_avals)
    return _CACHE["runner"]


def kernel(x, w_qkv, b_qkv, w_proj, b_proj):
    import jax
    fn, in_names, out_names, out_avals = _get_runner()
    per_core = {
        "x": [np.ascontiguousarray(x[i], dtype=np.float32) for i in range(N_CORES)],
        "w_qkv": [np.asarray(w_qkv, dtype=np.float32)] * N_CORES,
        "b_qkv": [np.asarray(b_qkv, dtype=np.float32)] * N_CORES,
        "w_proj": [np.asarray(w_proj, dtype=np.float32)] * N_CORES,
        "b_proj": [np.asarray(b_proj, dtype=np.float32)] * N_CORES,
    }
    concat_in = [np.concatenate(per_core[k], axis=0) for k in in_names]
    concat_zeros = [
        np.zeros((N_CORES * a.shape[0], *a.shape[1:]), a.dtype) for a in out_avals
    ]
    outs = fn(*concat_in, *concat_zeros)
    jax.block_until_ready(outs)
    oi = out_names.index("out")
    return np.asarray(outs[oi]).reshape(N_CORES, T, D).astype(np.float32)
